# revision 9
# baseline (speedup 1.0000x reference)
"""Trainium2 Bass kernel for the CaptionDecoder problem.

Data-parallel over batch: 64 batch items -> 8 cores x 8 items.
Per core: attention-LSTM recurrence over 31 timesteps with all big matmuls
in bf16 on the PE array, then one batched vocab projection streaming out_W.

Layout notes (per core, B=8 local batch):
 - n (spatial, 196) is host-padded to 256 so each batch item occupies exactly
   two 128-partition chunks; all attention block structure is chunk-aligned.
 - bn = b*256 + n is the fused row index (2048 rows = 16 chunks of 128).
 - fP   [bn, e]   features (padded rows zero)           - stage-A rhs
 - fT   [e, bn]   features transposed                   - enc_att rhs
 - enc_att S-layout: [a partition-chunks (4x128), b, n]  - tanh/e path
 - e/exp lands as [bn%128, chunk] via S-as-stationary matmuls, which is
   exactly the block-diagonal alpha (A) layout needed for stage A.
 - h, c, gates live transposed: [dec%128, dec//128, b].
"""

import sys, os
for _p in ("/opt/trn_rl_repo", os.path.dirname(os.path.abspath(__file__))):
    if _p not in sys.path:
        sys.path.insert(0, _p)

import numpy as np
import ml_dtypes

import concourse.bass as bass
import concourse.mybir as mybir
from concourse import bacc
from concourse.tile import TileContext
from concourse.bass_utils import run_bass_kernel_spmd
from concourse.bass import IndirectOffsetOnAxis
from concourse.masks import make_identity

F32 = mybir.dt.float32
BF16 = mybir.dt.bfloat16
I32 = mybir.dt.int32
AF = mybir.ActivationFunctionType
ALU = mybir.AluOpType

B, N, ENC, DEC, E, ATT, V, T = 64, 196, 2048, 512, 512, 512, 32000, 32
NCORES = 8
BL = B // NCORES          # 8 local batch
NP = 256                  # padded n
BN = BL * NP              # 2048
NCH = BN // 128           # 16 bn-chunks
ECH = ENC // 128          # 16 e-chunks
ACH = ATT // 128          # 4 a-chunks
DCH = DEC // 128          # 4 dec-chunks
TS = T - 1                # 31 timesteps

_CACHE = {}


def build(ts):
    tb = ts * BL
    mtb = [128, tb - 128] if tb > 128 else [tb]
    nc = bacc.Bacc("TRN2", target_bir_lowering=False, debug=False,
                   num_devices=NCORES)
    dram = lambda n_, s, d: nc.dram_tensor(n_, s, d, kind="ExternalInput").ap()
    out_ = lambda n_, s, d: nc.dram_tensor(n_, s, d, kind="ExternalOutput").ap()

    fP = dram("fP", [BN, ENC], BF16)
    fT = dram("fT", [ENC, BN], BF16)
    wenc = dram("wenc", [ENC, ATT], BF16)
    wdec = dram("wdec", [DEC, ATT], BF16)
    vcol = dram("vcol", [ATT, 1], BF16)
    hw = dram("hw", [ENC, DEC], BF16)
    cw = dram("cw", [ENC, DEC], BF16)
    hbias = dram("hbias", [128, DCH], F32)
    cbias = dram("cbias", [128, DCH], F32)
    wihe = dram("wihe", [E, 4 * DEC], BF16)
    wenc2 = dram("wenc2", [ENC, 4 * DEC], BF16)
    whh = dram("whh", [DEC, 4 * DEC], BF16)
    bih2 = dram("bih2", [128, 16], F32)
    emb = dram("emb", [V, E], F32)
    xidx = dram("xidx", [tb, 1], I32)
    ow = dram("ow", [DEC, V], BF16)
    obrep = dram("obrep", [128, V], BF16)

    preds = out_("preds", [BL, ts, V], F32)
    alph = out_("alph", [BL, ts, N], F32)

    asc = nc.dram_tensor("asc", [ts, 128, NCH], F32).ap()
    xgd = nc.dram_tensor("xgd", [ts, 128, 16, BL], BF16).ap()
    rzd = nc.dram_tensor("rzd", [1, tb], F32).ap()

    with TileContext(nc) as tc:
        with tc.tile_pool(name="sb", bufs=1) as sb:

            idb = sb.tile([128, 128], BF16, name="idb", tag="idb")
            make_identity(nc, idb[:])
            idf = sb.tile([128, 128], F32, name="idf", tag="idf")
            make_identity(nc, idf[:])

            # ---------- load features (both layouts) ----------
            fP_sb = sb.tile([128, NCH, ENC], BF16, name="fP_sb", tag="fP_sb")
            nc.sync.dma_start(out=fP_sb[:], in_=fP.rearrange("(c p) e -> p c e", p=128))
            pre1 = tc.alloc_tile_pool(name="pre1", bufs=1)
            ps1 = tc.alloc_tile_pool(name="ps1", bufs=1, space="PSUM")
            fT_sb = pre1.tile([128, ECH, BN], BF16, name="fT_sb", tag="fT_sb")
            nc.gpsimd.dma_start(out=fT_sb[:], in_=fT.rearrange("(c p) n -> p c n", p=128))


            # ---------- enc_att = (features @ att_Wenc).T -> [a, b, n] ----------
            wenc_sb = pre1.tile([128, ECH, ATT], BF16, name="wenc_sb", tag="wenc_sb")
            nc.sync.dma_start(out=wenc_sb[:], in_=wenc.rearrange("(c p) a -> p c a", p=128))
            att_sb = sb.tile([128, ACH, BL, NP], BF16, name="att_sb", tag="att_sb")
            for mc in range(ACH):
                for nb in range(4):  # bn in 4 chunks of 512 = 2 (b, half) units
                    p_att = ps1.tile([128, 2, NP], F32, name="p_att", tag="p_att", bufs=2)
                    for ec in range(ECH):
                        nc.tensor.matmul(out=p_att[:],
                                         lhsT=wenc_sb[:, ec, 128 * mc:128 * (mc + 1)],
                                         rhs=fT_sb[:, ec, 512 * nb:512 * (nb + 1)],
                                         start=(ec == 0), stop=(ec == ECH - 1))
                    nc.vector.tensor_copy(out=att_sb[:, mc, 2 * nb:2 * nb + 2, :],
                                          in_=p_att[:])

            ps1.release()
            pre1.release()
            pre2 = tc.alloc_tile_pool(name="pre2", bufs=1)
            ps2 = tc.alloc_tile_pool(name="ps2", bufs=1, space="PSUM")

            # ---------- mean pool -> mfT [e, b] ----------
            onesbd = pre2.tile([128, NCH, BL], BF16, name="onesbd", tag="onesbd")
            nc.vector.memset(onesbd[:], 0.0)
            for c in range(NCH):
                plen = 128 if c % 2 == 0 else N - 128
                nc.vector.memset(onesbd[0:plen, c, c // 2:c // 2 + 1], 1.0 / N)
            mean_sb = pre2.tile([8, ACH, 512], BF16, name="mean_sb", tag="mean_sb")
            for g in range(4):
                p_mean = ps2.tile([128, 512], F32, name="p_mean", tag="p_mean")
                for c in range(NCH):
                    nc.tensor.matmul(out=p_mean[32 * g:32 * g + BL, :],
                                     lhsT=onesbd[:, c, :],
                                     rhs=fP_sb[:, c, 512 * g:512 * (g + 1)],
                                     start=(c == 0), stop=(c == NCH - 1),
                                     tile_position=(0, 32 * g),
                                     skip_group_check=True)
                nc.vector.tensor_copy(out=mean_sb[0:BL, g, :], in_=p_mean[32 * g:32 * g + BL, :])
            mft = pre2.tile([128, ECH, BL], BF16, name="mft", tag="mft")
            for k in range(ECH):
                p_mft = ps2.tile([128, BL], BF16, name="p_mft", tag="p_mft", bufs=1)
                nc.tensor.transpose(out=p_mft[:],
                                    in_=mean_sb[0:BL, k // 4, 128 * (k % 4):128 * (k % 4 + 1)],
                                    identity=idb[0:BL, 0:BL])
                nc.vector.tensor_copy(out=mft[:, k, :], in_=p_mft[:])

            # ---------- h0 / c0 -> [dec, b] ----------
            hb_sb = sb.tile([128, DCH], F32, name="hb_sb", tag="hb_sb")
            nc.sync.dma_start(out=hb_sb[:], in_=hbias[:])
            cb_sb = sb.tile([128, DCH], F32, name="cb_sb", tag="cb_sb")
            nc.sync.dma_start(out=cb_sb[:], in_=cbias[:])
            h0b = sb.tile([128, DCH, BL], BF16, name="h0b", tag="h0b")
            cT = sb.tile([128, DCH, BL], F32, name="cT", tag="cT", bufs=2)
            for (wsrc, bsb, dst) in ((hw, hb_sb, h0b), (cw, cb_sb, cT)):
                w_sb = pre2.tile([128, ECH, DEC], BF16, name="w_sb", tag="w_sb", bufs=1)
                nc.sync.dma_start(out=w_sb[:], in_=wsrc.rearrange("(c p) d -> p c d", p=128))
                for mc in range(DCH):
                    p_h0 = ps2.tile([128, BL], F32, name="p_h0", tag="p_h0", bufs=1)
                    for ec in range(ECH):
                        nc.tensor.matmul(out=p_h0[:],
                                         lhsT=w_sb[:, ec, 128 * mc:128 * (mc + 1)],
                                         rhs=mft[:, ec, :],
                                         start=(ec == 0), stop=(ec == ECH - 1))
                    nc.scalar.activation(out=dst[:, mc, :], in_=p_h0[:],
                                         func=AF.Identity, bias=bsb[:, mc:mc + 1])

            # ---------- Xg (time-major embedding gate precursors) -> DRAM ----------
            xsT = pre2.tile([128, DCH, tb], BF16, name="xsT", tag="xsT")
            for m in range(len(mtb)):
                mlen = mtb[m]
                idx_t = pre2.tile([128, 1], I32, name="idx_t", tag="idx_t", bufs=2)
                nc.sync.dma_start(out=idx_t[0:mlen, :], in_=xidx[128 * m:128 * m + mlen, :])
                xs_f = pre2.tile([128, E], F32, name="xs_f", tag="xs_f", bufs=2)
                nc.gpsimd.indirect_dma_start(
                    out=xs_f[0:mlen, :], out_offset=None, in_=emb[:],
                    in_offset=IndirectOffsetOnAxis(ap=idx_t[0:mlen, :], axis=0))
                xs_b = pre2.tile([128, E], BF16, name="xs_b", tag="xs_b", bufs=2)
                nc.vector.tensor_copy(out=xs_b[0:mlen, :], in_=xs_f[0:mlen, :])
                for k in range(DCH):
                    p_xsT = ps2.tile([128, 128], BF16, name="p_xsT", tag="p_xsT", bufs=2)
                    nc.tensor.transpose(out=p_xsT[:, 0:mlen],
                                        in_=xs_b[0:mlen, 128 * k:128 * (k + 1)],
                                        identity=idb[0:mlen, 0:mlen])
                    nc.vector.tensor_copy(out=xsT[:, k, 128 * m:128 * m + mlen],
                                          in_=p_xsT[:, 0:mlen])
            wihe_sb = pre2.tile([128, DCH, 4 * DEC], BF16, name="wihe_sb", tag="wihe_sb")
            nc.sync.dma_start(out=wihe_sb[:], in_=wihe.rearrange("(c p) j -> p c j", p=128))
            bih_sb = pre2.tile([128, 16], F32, name="bih_sb", tag="bih_sb")
            nc.sync.dma_start(out=bih_sb[:], in_=bih2[:])
            for jm in range(16):
                p_xg = ps2.tile([128, ts, BL], F32, name="p_xg", tag="p_xg", bufs=2)
                for k in range(DCH):
                    nc.tensor.matmul(out=p_xg[:],
                                     lhsT=wihe_sb[:, k, 128 * jm:128 * (jm + 1)],
                                     rhs=xsT[:, k, :],
                                     start=(k == 0), stop=(k == DCH - 1))
                xg_st = pre2.tile([128, ts, BL], BF16, name="xg_st", tag="xg_st", bufs=2)
                nc.scalar.activation(out=xg_st[:], in_=p_xg[:],
                                     func=AF.Identity, bias=bih_sb[:, jm:jm + 1])
                nc.sync.dma_start(out=xgd[:, :, jm, :].rearrange("t p j -> p t j"),
                                  in_=xg_st[:])

            ps2.release()
            pre2.release()
            lp = tc.alloc_tile_pool(name="lp", bufs=1)
            psl = tc.alloc_tile_pool(name="psl", bufs=1, space="PSUM")


            # mask replicated as stationary for Z: two patterns (full / first 68)
            mrep = lp.tile([128, 2, 128], BF16, name="mrep", tag="mrep")
            nc.vector.memset(mrep[:], 0.0)
            nc.vector.memset(mrep[:, 0, :], 1.0)
            nc.vector.memset(mrep[0:N - 128, 1, :], 1.0)
            # ---------- loop-resident weights ----------
            wenc2_sb = lp.tile([128, ECH, 4 * DEC], BF16, name="wenc2_sb", tag="wenc2_sb")
            nc.sync.dma_start(out=wenc2_sb[:], in_=wenc2.rearrange("(c p) j -> p c j", p=128))
            whh_sb = lp.tile([128, DCH, 4 * DEC], BF16, name="whh_sb", tag="whh_sb")
            nc.sync.dma_start(out=whh_sb[:], in_=whh.rearrange("(c p) j -> p c j", p=128))
            wdec_sb = lp.tile([128, DCH, ATT], BF16, name="wdec_sb", tag="wdec_sb")
            nc.sync.dma_start(out=wdec_sb[:], in_=wdec.rearrange("(c p) a -> p c a", p=128))
            vc_sb = lp.tile([128, ACH], BF16, name="vc_sb", tag="vc_sb")
            nc.sync.dma_start(out=vc_sb[:], in_=vcol.rearrange("(c p) o -> p (c o)", p=128))

            A_sb = lp.tile([128, NCH, BL], BF16, name="A_sb", tag="A_sb")
            nc.vector.memset(A_sb[:], 0.0)
            S_sb = lp.tile([128, ACH, BL, NP], BF16, name="S_sb", tag="S_sb")
            Hb = sb.tile([128, DCH, tb], BF16, name="Hb", tag="Hb")
            rzall = sb.tile([128, tb], F32, name="rzall", tag="rzall")

            # ---------- recurrence ----------
            for t in range(ts):
                hsl = h0b if t == 0 else Hb
                hoff = 0 if t == 0 else (t - 1) * BL

                # d = (h @ att_Wdec).T  -> [a, b]
                p_d = psl.tile([128, ACH, BL, 1], F32, name="p_d", tag="p_d")
                for mc in range(ACH):
                    for kc in range(DCH):
                        nc.tensor.matmul(out=p_d[:, mc, :, :],
                                         lhsT=wdec_sb[:, kc, 128 * mc:128 * (mc + 1)],
                                         rhs=hsl[:, kc, hoff:hoff + BL],
                                         start=(kc == 0), stop=(kc == DCH - 1),
                                         skip_group_check=True)
                d_b = lp.tile([128, ACH, BL, 1], BF16, name="d_b", tag="d_b")
                nc.vector.tensor_copy(out=d_b[:], in_=p_d[:])

                # S = tanh(enc_att + d); e^T via S-as-stationary matmuls.
                # Interleaved per-column PSUM accumulation is broken on HW, so
                # each (mc, s) matmul writes its own psum column; DVE reduces.
                p_e = psl.tile([128, NCH, ACH], F32, name="p_e", tag="p_e")
                for mc in range(ACH):
                    nc.vector.tensor_tensor(out=S_sb[:, mc, :, :], in0=att_sb[:, mc, :, :],
                                            in1=d_b[:, mc, :, :].to_broadcast([128, BL, NP]),
                                            op=ALU.add)
                    nc.scalar.activation(out=S_sb[:, mc, :, :], in_=S_sb[:, mc, :, :],
                                         func=AF.Tanh)
                    for s in range(NCH):
                        nc.tensor.matmul(out=p_e[:, s, mc:mc + 1],
                                         lhsT=S_sb[:, mc, s // 2, (s % 2) * 128:(s % 2) * 128 + 128],
                                         rhs=vc_sb[:, mc:mc + 1],
                                         start=True, stop=True,
                                         skip_group_check=True)

                # exp (unnormalized alpha) -> E (f32) and block-diag A (bf16)
                e_sb = lp.tile([128, NCH], F32, name="e_sb", tag="e_sb")
                nc.vector.tensor_reduce(out=e_sb[:], in_=p_e[:], axis=mybir.AxisListType.X,
                                        op=ALU.add)
                Ef = lp.tile([128, NCH], F32, name="Ef", tag="Ef", bufs=2)
                nc.scalar.activation(out=Ef[:], in_=e_sb[:], func=AF.Exp)
                for q in range(BL):
                    nc.vector.tensor_copy(out=A_sb[:, 2 * q:2 * q + 2, q],
                                          in_=Ef[:, 2 * q:2 * q + 2])
                nc.sync.dma_start(out=asc[t], in_=Ef[:])

                # Z replicated across partitions -> rz [p, b]
                p_zr = psl.tile([128, BL], F32, name="p_zr", tag="p_zr")
                for c in range(NCH):
                    nc.tensor.matmul(out=p_zr[:], lhsT=mrep[:, c % 2, :],
                                     rhs=A_sb[:, c, :],
                                     start=(c == 0), stop=(c == NCH - 1),
                                     skip_group_check=True)
                rzr = lp.tile([128, 1, BL], F32, name="rzr", tag="rzr")
                nc.vector.reciprocal(out=rzr[:, 0, :], in_=p_zr[:])
                nc.vector.tensor_copy(out=rzall[:, t * BL:(t + 1) * BL], in_=rzr[:, 0, :])

                # stage A: ctx = A.T @ fP (col-tiled over 4 e-groups)
                p_ctx = psl.tile([128, 512], F32, name="p_ctx", tag="p_ctx")
                for c in range(NCH):
                    for g in range(4):
                        nc.tensor.matmul(out=p_ctx[32 * g:32 * g + BL, :],
                                         lhsT=A_sb[:, c, :],
                                         rhs=fP_sb[:, c, 512 * g:512 * (g + 1)],
                                         start=(c == 0), stop=(c == NCH - 1),
                                         tile_position=(0, 32 * g),
                                         skip_group_check=True)
                ctx_sb = lp.tile([8, 4, 512], BF16, name="ctx_sb", tag="ctx_sb")
                for g in range(4):
                    nc.vector.tensor_copy(out=ctx_sb[0:BL, g, :],
                                          in_=p_ctx[32 * g:32 * g + BL, :])
                # transpose ctx -> [e, b]; scale by 1/Z on evac
                p_cT = psl.tile([128, ECH, BL], BF16, name="p_cT", tag="p_cT")
                for k in range(ECH):
                    nc.tensor.transpose(out=p_cT[:, k, :],
                                        in_=ctx_sb[0:BL, k // 4, 128 * (k % 4):128 * (k % 4 + 1)],
                                        identity=idb[0:BL, 0:BL])
                cxT = lp.tile([128, ECH, BL], BF16, name="cxT", tag="cxT")
                nc.vector.tensor_tensor(out=cxT[:], in0=p_cT[:],
                                        in1=rzr[:].to_broadcast([128, ECH, BL]),
                                        op=ALU.mult)

                # gates: ctx@W_ih_enc.T + h@W_hh.T (col-tiled over 4 j-groups)
                p_g = psl.tile([128, 512], F32, name="p_g", tag="p_g")
                for g in range(4):
                    for kc in range(ECH):
                        nc.tensor.matmul(out=p_g[32 * g:32 * g + BL, :],
                                         lhsT=cxT[:, kc, :],
                                         rhs=wenc2_sb[:, kc, 512 * g:512 * (g + 1)],
                                         start=(kc == 0), stop=False,
                                         tile_position=(0, 32 * g),
                                         skip_group_check=True)
                    for kc in range(DCH):
                        nc.tensor.matmul(out=p_g[32 * g:32 * g + BL, :],
                                         lhsT=hsl[:, kc, hoff:hoff + BL],
                                         rhs=whh_sb[:, kc, 512 * g:512 * (g + 1)],
                                         start=False, stop=(kc == DCH - 1),
                                         tile_position=(0, 32 * g),
                                         skip_group_check=True)
                gsb = lp.tile([8, 4, 512], BF16, name="gsb", tag="gsb")
                for g in range(4):
                    nc.vector.tensor_copy(out=gsb[0:BL, g, :],
                                          in_=p_g[32 * g:32 * g + BL, :])
                p_gT = psl.tile([128, 16, BL], BF16, name="p_gT", tag="p_gT")
                for k in range(16):
                    nc.tensor.transpose(out=p_gT[:, k, :],
                                        in_=gsb[0:BL, k // 4, 128 * (k % 4):128 * (k % 4 + 1)],
                                        identity=idb[0:BL, 0:BL])
                xgt = lp.tile([128, 16, BL], BF16, name="xgt", tag="xgt", bufs=3)
                nc.gpsimd.dma_start(out=xgt[:], in_=xgd[t])
                gt = lp.tile([128, 16, BL], F32, name="gt", tag="gt")
                nc.vector.tensor_tensor(out=gt[:], in0=p_gT[:], in1=xgt[:], op=ALU.add)

                # nonlinearities + cell update ([dec%128, dec//128, b])
                sif = lp.tile([128, 2 * DCH, BL], F32, name="sif", tag="sif")
                nc.scalar.activation(out=sif[:], in_=gt[:, 0:8, :], func=AF.Sigmoid)
                tg = lp.tile([128, DCH, BL], F32, name="tg", tag="tg")
                nc.scalar.activation(out=tg[:], in_=gt[:, 8:12, :], func=AF.Tanh)
                so = lp.tile([128, DCH, BL], F32, name="so", tag="so")
                nc.scalar.activation(out=so[:], in_=gt[:, 12:16, :], func=AF.Sigmoid)
                tmp = lp.tile([128, DCH, BL], F32, name="tmp", tag="tmp")
                nc.vector.tensor_tensor(out=tmp[:], in0=sif[:, 0:4, :], in1=tg[:], op=ALU.mult)
                cN = lp.tile([128, DCH, BL], F32, name="cN", tag="cT", bufs=2)
                nc.vector.tensor_tensor(out=cN[:], in0=sif[:, 4:8, :], in1=cT[:], op=ALU.mult)
                nc.vector.tensor_tensor(out=cN[:], in0=cN[:], in1=tmp[:], op=ALU.add)
                tc_ = lp.tile([128, DCH, BL], F32, name="tc_", tag="tc_")
                nc.scalar.activation(out=tc_[:], in_=cN[:], func=AF.Tanh)
                nc.vector.tensor_tensor(out=Hb[:, :, t * BL:(t + 1) * BL],
                                        in0=so[:], in1=tc_[:], op=ALU.mult)
                cT = cN

            psl.release()
            lp.release()
            fin = tc.alloc_tile_pool(name="fin", bufs=1)
            psf = tc.alloc_tile_pool(name="psf", bufs=1, space="PSUM")

            # ---------- alphas: normalize + transpose to [t, b, n] ----------
            nc.sync.dma_start(out=rzd[:], in_=rzall[0:1, :])
            Eall = fin.tile([128, ts, NCH], F32, name="Eall", tag="Eall")
            nc.sync.dma_start(out=Eall[:], in_=asc.rearrange("t p c -> p t c"))
            rzT31 = fin.tile([ts, BL], F32, name="rzT31", tag="rzT31")
            nc.sync.dma_start(out=rzT31[:], in_=rzd.rearrange("o (t b) -> (o t) b", b=BL))
            alphT = fin.tile([ts, BL, N], F32, name="alphT", tag="alphT")
            for c in range(NCH):
                plen = 128 if c % 2 == 0 else N - 128
                p_tr = psf.tile([128, 128], F32, name="p_tr", tag="p_tr", bufs=2)
                nc.tensor.transpose(out=p_tr[0:ts, :], in_=Eall[:, :, c],
                                    identity=idf[:])
                nc.vector.tensor_tensor(
                    out=alphT[0:ts, c // 2, (c % 2) * 128:(c % 2) * 128 + plen],
                    in0=p_tr[0:ts, 0:plen],
                    in1=rzT31[0:ts, c // 2:c // 2 + 1].to_broadcast([ts, plen]),
                    op=ALU.mult)
            nc.sync.dma_start(out=alph.rearrange("b t n -> t b n"), in_=alphT[0:ts, :, :])

            # ---------- vocab projection ----------
            VC = 1024
            vchunks = [(i * VC, VC) for i in range(V // VC)]
            if V % VC:
                vchunks.append((V - V % VC, V % VC))
            for (voff, vlen) in vchunks:
                owt = fin.tile([128, DCH, VC], BF16, name="owt", tag="owt", bufs=2)
                nc.sync.dma_start(out=owt[:, :, 0:vlen],
                                  in_=ow[:, voff:voff + vlen].rearrange("(c p) v -> p c v", p=128))
                obt = fin.tile([128, VC], BF16, name="obt", tag="obt", bufs=2)
                nc.gpsimd.dma_start(out=obt[:, 0:vlen], in_=obrep[:, voff:voff + vlen])
                for sub in range(0, vlen, 512):
                    sl = min(512, vlen - sub)
                    for m in range(len(mtb)):
                        mlen = mtb[m]
                        p_pr = psf.tile([128, 512], F32, name="p_pr", tag="p_pr", bufs=2)
                        for dc in range(DCH):
                            nc.tensor.matmul(out=p_pr[0:mlen, 0:sl],
                                             lhsT=Hb[:, dc, 128 * m:128 * m + mlen],
                                             rhs=owt[:, dc, sub:sub + sl],
                                             start=(dc == 0), stop=(dc == DCH - 1),
                                             skip_group_check=True)
                        pr_sb = fin.tile([128, 512], F32, name="pr_sb", tag="pr_sb", bufs=3)
                        nc.vector.tensor_tensor(out=pr_sb[0:mlen, 0:sl],
                                                in0=p_pr[0:mlen, 0:sl],
                                                in1=obt[0:mlen, sub:sub + sl],
                                                op=ALU.add)
                        nc.sync.dma_start(
                            out=preds.rearrange("b t v -> t b v")[16 * m:16 * m + mlen // BL,
                                                                  :, voff + sub:voff + sub + sl],
                            in_=pr_sb[0:mlen, 0:sl])
            psf.release()
            fin.release()
    nc.compile()
    return nc


def _prep(inputs, ts):
    bf = lambda a: np.ascontiguousarray(np.asarray(a)).astype(ml_dtypes.bfloat16)
    f32 = lambda a: np.ascontiguousarray(np.asarray(a), dtype=np.float32)
    feats = f32(inputs["features"])
    caps = np.asarray(inputs["captions"]).astype(np.int32)
    W_ih = f32(inputs["W_ih"])
    shared = {
        "wenc": bf(inputs["att_Wenc"]),
        "wdec": bf(inputs["att_Wdec"]),
        "vcol": bf(np.asarray(inputs["att_v"]).reshape(ATT, 1)),
        "hw": bf(inputs["init_h_W"]),
        "cw": bf(inputs["init_c_W"]),
        "hbias": f32(np.asarray(inputs["init_h_b"]).reshape(DCH, 128).T),
        "cbias": f32(np.asarray(inputs["init_c_b"]).reshape(DCH, 128).T),
        "wihe": bf(W_ih[:, :E].T),
        "wenc2": bf(W_ih[:, E:].T),
        "whh": bf(f32(inputs["W_hh"]).T),
        "bih2": f32((f32(inputs["b_ih"]) + f32(inputs["b_hh"])).reshape(16, 128).T),
        "emb": f32(inputs["emb"]),
        "ow": bf(inputs["out_W"]),
        "obrep": bf(np.broadcast_to(np.asarray(inputs["out_b"]).reshape(1, V), (128, V))),
    }
    in_maps = []
    for c in range(NCORES):
        fb = feats[BL * c:BL * (c + 1)]          # [8, 196, 2048]
        fPn = np.zeros((BL, NP, ENC), np.float32)
        fPn[:, :N, :] = fb
        fPn = fPn.reshape(BN, ENC)
        xi = caps[BL * c:BL * (c + 1), :ts].T.reshape(ts * BL, 1)  # (t, b) major
        m = dict(shared)
        m["fP"] = bf(fPn)
        m["fT"] = bf(fPn.T)
        m["xidx"] = np.ascontiguousarray(xi)
        in_maps.append(m)
    return in_maps


def _run(inputs, ts, **kw):
    if ts not in _CACHE:
        _CACHE[ts] = build(ts)
    nc = _CACHE[ts]
    in_maps = _prep(inputs, ts)
    res = run_bass_kernel_spmd(nc, in_maps, list(range(NCORES)), **kw)
    preds = np.empty((B, ts, V), np.float32)
    alphas = np.empty((B, ts, N), np.float32)
    for c in range(NCORES):
        preds[BL * c:BL * (c + 1)] = res.results[c]["preds"]
        alphas[BL * c:BL * (c + 1)] = res.results[c]["alph"]
    return preds, alphas, res


def kernel(**inputs):
    preds, alphas, _ = _run(inputs, TS)
    return preds, alphas
